# revision 28
# baseline (speedup 1.0000x reference)
"""Additive attention (Bahdanau) Trainium2 kernel.

Problem (hardcoded):
    query  [32, 1, 1024] f32
    keys   [32, 4096, 1024] f32
    values [32, 4096, 1024] f32
    mask   [32, 4096] bool
    Wq, Wk [1024, 1024] f32, v [1024] f32

    q = query @ Wq;  k = keys @ Wk
    scores = v . tanh(q + k);  scores = where(mask, scores, -inf)
    attn = softmax(scores);  ctx = attn @ values
    returns (ctx, attn)

Sharding: data-parallel over batch, 4 batches per core on 8 NeuronCores.

Per-core layout ("[a, t]" layout):
  - keys chunks are loaded [128p, c, h] with DRAM row = C*p + c so each
    partition reads one contiguous 16KB run (big DMA descriptors). Within
    a t-chunk the t order is therefore permuted: stored index (c, j)
    corresponds to t = t0 + C*j + c. The permutation is self-consistent
    through scores -> p -> pcol -> ctx (values use the same load layout);
    only the mask read and the attn store un-permute via strided APs.
  - keys tiles [128t, 128h] are PE-transposed to [128h, t] so the H
    contraction sits on partitions.
  - K^T[a, t] = Wk[h,a]^T @ keysT[h,t] accumulated over h-chunks in PSUM.
  - tanh on ScalarE reads PSUM with per-partition bias q[a] (fused add).
  - scores^T[1, t] = v_chunk[128a,1]^T @ g[128a, t] accumulated over a.
  - exp on ScalarE -> p[1, Tk] (no max subtraction needed: |scores|<=32),
    masked by a VectorE multiply.
  - p moved to [128t, Tk/128] via K=1 matmuls; row-reduce + ones-matmul
    give l; ctx = p^T @ values accumulated in PSUM, scaled by 1/l at
    evacuation; attn = p * 1/l on VectorE.
"""

import numpy as np

import concourse.bass as bass
import concourse.tile as tile
from concourse import bacc, mybir
from concourse.bass_utils import run_bass_kernel_spmd
from concourse.masks import make_identity

F32 = mybir.dt.float32
F32R = mybir.dt.float32r
BF16 = mybir.dt.bfloat16
U8 = mybir.dt.uint8
AF = mybir.ActivationFunctionType

# Full problem constants
B, TQ, TK, H, A = 32, 1, 4096, 1024, 1024
N_CORES = 8
B_LOCAL = B // N_CORES

# dtype of the keys-projection matmul path: "bf16" | "f32r" | "f32"
MM_MODE = "bf16"
# dtype of the ctx (attn @ values) matmul path
VAL_DT = F32R

_DT = {"bf16": BF16, "f32r": F32R, "f32": F32}


def build_nc(b_local=B_LOCAL, tk=TK, h=H, a=A, tch=512, mm_mode=None):
    """Build the per-core Bass module (SPMD: same program each core)."""
    mm_dt = _DT[mm_mode or MM_MODE]
    P = 128
    HC = h // P          # h chunks
    AC = a // P          # a chunks
    NTCH = tk // tch     # t chunks
    C = tch // P         # 128-blocks per t chunk
    NJ = tk // P         # t 128-blocks per batch

    nc = bacc.Bacc("TRN2", target_bir_lowering=False, debug=False)

    keys_d = nc.dram_tensor("keys", [b_local, tk, h], F32, kind="ExternalInput")
    values_d = nc.dram_tensor("values", [b_local, tk, h], F32, kind="ExternalInput")
    query_d = nc.dram_tensor("query", [b_local, h], F32, kind="ExternalInput")
    mask_d = nc.dram_tensor("mask", [b_local, tk], U8, kind="ExternalInput")
    wq_d = nc.dram_tensor("Wq", [h, a], F32, kind="ExternalInput")
    wk_d = nc.dram_tensor("Wk", [h, a], F32, kind="ExternalInput")
    v_d = nc.dram_tensor("v", [a], F32, kind="ExternalInput")

    ctx_d = nc.dram_tensor("ctx", [b_local, h], F32, kind="ExternalOutput")
    attn_d = nc.dram_tensor("attn", [b_local, tk], F32, kind="ExternalOutput")

    with tile.TileContext(nc) as tc:
        with (
            tc.tile_pool(name="const", bufs=1) as const_pool,
            tc.tile_pool(name="wq", bufs=2) as wq_pool,
            tc.tile_pool(name="kraw", bufs=3) as kraw_pool,
            tc.tile_pool(name="kt", bufs=2) as kt_pool,
            tc.tile_pool(name="g", bufs=4) as g_pool,
            tc.tile_pool(name="val", bufs=3) as val_pool,
            tc.tile_pool(name="small", bufs=1) as small_pool,
            tc.tile_pool(name="psum", bufs=1, space="PSUM") as psum_pool,
        ):
            # ---------------- constants ----------------
            ident = const_pool.tile([P, P], F32)
            make_identity(nc, ident)
            ones_col = const_pool.tile([P, 1], F32)
            nc.vector.memset(ones_col, 1.0)
            one_sc = const_pool.tile([1, 1], F32)
            nc.vector.memset(one_sc, 1.0)

            # Wk resident in SBUF: [128p, hc, a]  (p is h mod 128)
            wk_sb = const_pool.tile([P, HC, a], mm_dt)
            nc.gpsimd.dma_start(wk_sb, wk_d.rearrange("(hc p) a -> p hc a", p=P))

            # v as [128p, ac]
            v_sb = const_pool.tile([P, AC], mm_dt)
            with nc.allow_non_contiguous_dma(reason="tiny 4KB v load"):
                nc.gpsimd.dma_start(v_sb, v_d.rearrange("(ac p) -> p ac", p=P))

            # queryT: [128p, hc*b] built via PE transpose of [b, 128] blocks
            qraw_sb = const_pool.tile([b_local, h], F32)
            nc.sync.dma_start(qraw_sb, query_d[:, :])
            qt_ps = psum_pool.tile([P, HC * b_local], F32, tag="sc", bufs=2,
                                   name="qt_ps")
            for hc in range(HC):
                nc.tensor.transpose(
                    qt_ps[:, hc * b_local : (hc + 1) * b_local],
                    qraw_sb[:, hc * P : (hc + 1) * P],
                    ident[0:b_local, 0:b_local],
                )
            qt_sb = const_pool.tile([P, HC * b_local], F32)
            nc.vector.tensor_copy(qt_sb, qt_ps)

            # q^T[b, a] = query @ Wq, accumulated over hc.
            q_bt_ps = psum_pool.tile([b_local, a], F32, tag="ctx", bufs=1,
                                     name="q_bt_ps")
            for hc in range(HC):
                wq_sb = wq_pool.tile([P, a], F32)
                nc.sync.dma_start(
                    wq_sb, wq_d.rearrange("(hc p) a -> p hc a", p=P)[:, hc]
                )
                for ha in range(a // 512):
                    nc.tensor.matmul(
                        q_bt_ps[:, ha * 512 : (ha + 1) * 512],
                        lhsT=qt_sb[:, hc * b_local : (hc + 1) * b_local],
                        rhs=wq_sb[:, ha * 512 : (ha + 1) * 512],
                        start=(hc == 0),
                        stop=(hc == HC - 1),
                    )
            q_bt_sb = const_pool.tile([b_local, a], F32)
            nc.scalar.copy(q_bt_sb, q_bt_ps)
            # transpose q^T [b, a] -> q columns [128p(a), ac*b]
            qcol_ps = psum_pool.tile([P, AC * b_local], F32, tag="sc", bufs=2,
                                     name="qcol_ps")
            for ac in range(AC):
                nc.tensor.transpose(
                    qcol_ps[:, ac * b_local : (ac + 1) * b_local],
                    q_bt_sb[:, ac * P : (ac + 1) * P],
                    ident[0:b_local, 0:b_local],
                )
            q_sb = const_pool.tile([P, AC * b_local], F32)
            nc.vector.tensor_copy(q_sb, qcol_ps)

            # ---------------- main loop over local batches ----------------
            for b in range(b_local):
                # p (unnormalized masked softmax numerator) [1, tk],
                # per-chunk entries permuted: idx (c,j) <-> t = t0 + C*j + c
                p_sb = small_pool.tile([1, tk], F32, tag="p")
                masku8_sb = small_pool.tile([1, tk], U8, tag="masku8", bufs=2)
                nc.sync.dma_start(masku8_sb, mask_d[b][None, :])
                # p columns [128t, NJ] for the ctx contraction
                pcol_sb = small_pool.tile([P, NJ], VAL_DT, tag="pcol", bufs=2)
                # ctx accumulator [1, h] lives across the whole batch
                ctx_ps = psum_pool.tile([1, h], F32, tag="ctx", bufs=1)

                for tci in range(NTCH):
                    t0 = tci * tch
                    # keys chunk [128p, c, h]; DRAM row = t0 + C*p + c so a
                    # partition reads one contiguous C*h*4B run
                    kraw = kraw_pool.tile([P, C, h], F32)
                    nc.sync.dma_start(
                        kraw,
                        keys_d[b, t0 : t0 + tch].rearrange(
                            "(p c) h -> p c h", c=C
                        ),
                    )
                    # values chunk (same permuted layout), streams on SWDGE
                    val = val_pool.tile([P, C, h], VAL_DT)
                    nc.gpsimd.dma_start(
                        val,
                        values_d[b, t0 : t0 + tch].rearrange(
                            "(p c) h -> p c h", c=C
                        ),
                    )
                    # transpose keys to keysT [128p(h), hc, tch]
                    kt = kt_pool.tile([P, HC, tch], mm_dt)
                    for hc in range(HC):
                        tr_ps = psum_pool.tile([P, tch], F32, tag="tr",
                                               bufs=2)
                        for c in range(C):
                            nc.tensor.transpose(
                                tr_ps[:, c * P : (c + 1) * P],
                                kraw[:, c, hc * P : (hc + 1) * P],
                                ident,
                            )
                        nc.vector.tensor_copy(kt[:, hc], tr_ps)

                    # scores^T accumulate [1, tch]; the v^T @ g matmul for
                    # chunk ac is emitted after the main matmuls of ac+1 so
                    # the PE never waits on the ScalarE tanh.
                    sc_ps = psum_pool.tile([1, tch], F32, tag="sc", bufs=2)
                    g_prev = None
                    for ac in range(AC):
                        mm_ps = psum_pool.tile([P, tch], F32, tag="mm", bufs=2)
                        for hc in range(HC):
                            nc.tensor.matmul(
                                mm_ps,
                                lhsT=wk_sb[:, hc, ac * P : (ac + 1) * P],
                                rhs=kt[:, hc],
                                start=(hc == 0),
                                stop=(hc == HC - 1),
                            )
                        g = g_pool.tile([P, tch], mm_dt)
                        nc.scalar.activation(
                            g, mm_ps, AF.Tanh,
                            bias=q_sb[:, ac * b_local + b : ac * b_local + b + 1],
                        )
                        if g_prev is not None:
                            nc.tensor.matmul(
                                sc_ps,
                                lhsT=v_sb[:, ac - 1 : ac],
                                rhs=g_prev,
                                start=(ac - 1 == 0),
                                stop=False,
                            )
                        g_prev = g
                    nc.tensor.matmul(
                        sc_ps,
                        lhsT=v_sb[:, AC - 1 : AC],
                        rhs=g_prev,
                        start=False,
                        stop=True,
                    )
                    # p = exp(scores) * mask (mask read un-permutes)
                    nc.scalar.activation(
                        p_sb[:, t0 : t0 + tch], sc_ps, AF.Exp
                    )
                    maskf = small_pool.tile([1, tch], F32, tag="maskf", bufs=2)
                    nc.vector.tensor_copy(
                        maskf.rearrange("o (c j) -> o c j", c=C),
                        masku8_sb[:, t0 : t0 + tch].rearrange(
                            "o (j c) -> o c j", c=C
                        ),
                    )
                    nc.vector.tensor_mul(
                        p_sb[:, t0 : t0 + tch], p_sb[:, t0 : t0 + tch], maskf
                    )

                    # p chunk -> pcol columns via K=1 matmuls
                    pcol_ps = psum_pool.tile([P, C], F32, tag="sc", bufs=2)
                    for c in range(C):
                        nc.tensor.matmul(
                            pcol_ps[:, c : c + 1],
                            lhsT=p_sb[:, t0 + c * P : t0 + (c + 1) * P],
                            rhs=one_sc,
                            start=(c == 0),
                            stop=(c == C - 1),
                        )
                    nc.vector.tensor_copy(
                        pcol_sb[:, tci * C : (tci + 1) * C], pcol_ps
                    )

                    # ctx += p_chunk^T @ values_chunk (unnormalized)
                    for c in range(C):
                        j = tci * C + c
                        for hh in range(h // 512):
                            nc.tensor.matmul(
                                ctx_ps[:, hh * 512 : (hh + 1) * 512],
                                lhsT=pcol_sb[:, j : j + 1],
                                rhs=val[:, c, hh * 512 : (hh + 1) * 512],
                                start=(j == 0),
                                stop=(j == NJ - 1),
                            )

                # ---- batch epilogue: l, outputs ----
                rowsum = small_pool.tile([P, 1], F32, tag="rowsum", bufs=2)
                nc.vector.tensor_reduce(
                    rowsum, pcol_sb, mybir.AxisListType.X, mybir.AluOpType.add
                )
                l_ps = psum_pool.tile([1, 1], F32, tag="sc", bufs=2)
                nc.tensor.matmul(l_ps, lhsT=rowsum, rhs=ones_col,
                                 start=True, stop=True)
                linv_sb = small_pool.tile([1, 1], F32, tag="linv", bufs=2)
                nc.vector.reciprocal(linv_sb, l_ps)

                # attn output = p * linv (in-place), DVE un-permute, store
                nc.vector.tensor_scalar_mul(p_sb, p_sb, linv_sb)
                attn_sb = small_pool.tile([1, tk], F32, tag="attn", bufs=1)
                nc.vector.tensor_copy(
                    attn_sb.rearrange("o (tc j c) -> o tc j c", c=C, j=P),
                    p_sb.rearrange("o (tc c j) -> o tc j c", c=C, j=P),
                )
                nc.sync.dma_start(attn_d[b][None, :], attn_sb)

                ctx_sb = small_pool.tile([1, h], F32, tag="ctx")
                nc.scalar.activation(ctx_sb, ctx_ps, AF.Copy, scale=linv_sb)
                nc.sync.dma_start(ctx_d[b][None, :], ctx_sb)

    nc.compile()
    return nc


LAST_RESULT = None


def kernel(query, keys, values, mask, Wq, Wk, v, run_kwargs=None):
    global LAST_RESULT
    query = np.asarray(query, dtype=np.float32)
    keys = np.asarray(keys, dtype=np.float32)
    values = np.asarray(values, dtype=np.float32)
    mask_u8 = np.asarray(mask).astype(np.uint8)
    Wq = np.ascontiguousarray(np.asarray(Wq, dtype=np.float32))
    Wk = np.ascontiguousarray(np.asarray(Wk, dtype=np.float32))
    v = np.ascontiguousarray(np.asarray(v, dtype=np.float32))

    nc = build_nc()

    in_maps = []
    for ci in range(N_CORES):
        sl = slice(ci * B_LOCAL, (ci + 1) * B_LOCAL)
        in_maps.append(
            {
                "keys": np.ascontiguousarray(keys[sl]),
                "values": np.ascontiguousarray(values[sl]),
                "query": np.ascontiguousarray(query[sl, 0]),
                "mask": np.ascontiguousarray(mask_u8[sl]),
                "Wq": Wq,
                "Wk": Wk,
                "v": v,
            }
        )

    res = run_bass_kernel_spmd(
        nc, in_maps, core_ids=list(range(N_CORES)), **(run_kwargs or {})
    )
    LAST_RESULT = res
    ctx = np.concatenate([r["ctx"] for r in res.results], axis=0)
    attn = np.concatenate([r["attn"] for r in res.results], axis=0)
    return ctx.reshape(B, TQ, H), attn.reshape(B, TQ, TK)


# revision 29
# speedup vs baseline: 1.1858x; 1.1858x over previous
"""Additive attention (Bahdanau) Trainium2 kernel.

Problem (hardcoded):
    query  [32, 1, 1024] f32
    keys   [32, 4096, 1024] f32
    values [32, 4096, 1024] f32
    mask   [32, 4096] bool
    Wq, Wk [1024, 1024] f32, v [1024] f32

    q = query @ Wq;  k = keys @ Wk
    scores = v . tanh(q + k);  scores = where(mask, scores, -inf)
    attn = softmax(scores);  ctx = attn @ values
    returns (ctx, attn)

Sharding: data-parallel over batch, 4 batches per core on 8 NeuronCores.

Per-core layout ("[a, t]" layout):
  - keys chunks are loaded [128p, c, h] with DRAM row = C*p + c so each
    partition reads one contiguous 16KB run (big DMA descriptors). Within
    a t-chunk the t order is therefore permuted: stored index (c, j)
    corresponds to t = t0 + C*j + c. The permutation is self-consistent
    through scores -> p -> pcol -> ctx (values use the same load layout);
    only the mask read and the attn store un-permute via strided APs.
  - keys tiles [128t, 128h] are PE-transposed to [128h, t] so the H
    contraction sits on partitions.
  - K^T[a, t] = Wk[h,a]^T @ keysT[h,t] accumulated over h-chunks in PSUM.
  - tanh on ScalarE reads PSUM with per-partition bias q[a] (fused add).
  - scores^T[1, t] = v_chunk[128a,1]^T @ g[128a, t] accumulated over a.
  - exp on ScalarE -> p[1, Tk] (no max subtraction needed: |scores|<=32),
    masked by a VectorE multiply.
  - p moved to [128t, Tk/128] via K=1 matmuls; row-reduce + ones-matmul
    give l; ctx = p^T @ values accumulated in PSUM, scaled by 1/l at
    evacuation; attn = p * 1/l on VectorE.
"""

import numpy as np

import concourse.bass as bass
import concourse.tile as tile
from concourse import bacc, mybir
from concourse.bass_utils import run_bass_kernel_spmd
from concourse.masks import make_identity

F32 = mybir.dt.float32
F32R = mybir.dt.float32r
BF16 = mybir.dt.bfloat16
U8 = mybir.dt.uint8
AF = mybir.ActivationFunctionType

# Full problem constants
B, TQ, TK, H, A = 32, 1, 4096, 1024, 1024
N_CORES = 8
B_LOCAL = B // N_CORES

# dtype of the keys-projection matmul path: "bf16" | "f32r" | "f32"
MM_MODE = "bf16"
# dtype of the ctx (attn @ values) matmul path
VAL_DT = F32R

_DT = {"bf16": BF16, "f32r": F32R, "f32": F32}


def build_nc(b_local=B_LOCAL, tk=TK, h=H, a=A, tch=512, mm_mode=None):
    """Build the per-core Bass module (SPMD: same program each core)."""
    mm_dt = _DT[mm_mode or MM_MODE]
    P = 128
    HC = h // P          # h chunks
    AC = a // P          # a chunks
    NTCH = tk // tch     # t chunks
    C = tch // P         # 128-blocks per t chunk
    NJ = tk // P         # t 128-blocks per batch

    nc = bacc.Bacc("TRN2", target_bir_lowering=False, debug=False)

    keys_d = nc.dram_tensor("keys", [b_local, tk, h], F32, kind="ExternalInput")
    values_d = nc.dram_tensor("values", [b_local, tk, h], F32, kind="ExternalInput")
    query_d = nc.dram_tensor("query", [b_local, h], F32, kind="ExternalInput")
    mask_d = nc.dram_tensor("mask", [b_local, tk], U8, kind="ExternalInput")
    wq_d = nc.dram_tensor("Wq", [h, a], F32, kind="ExternalInput")
    wk_d = nc.dram_tensor("Wk", [h, a], F32, kind="ExternalInput")
    v_d = nc.dram_tensor("v", [a], F32, kind="ExternalInput")

    ctx_d = nc.dram_tensor("ctx", [b_local, h], F32, kind="ExternalOutput")
    attn_d = nc.dram_tensor("attn", [b_local, tk], F32, kind="ExternalOutput")

    with tile.TileContext(nc) as tc:
        with (
            tc.tile_pool(name="const", bufs=1) as const_pool,
            tc.tile_pool(name="wq", bufs=2) as wq_pool,
            tc.tile_pool(name="kraw", bufs=3) as kraw_pool,
            tc.tile_pool(name="kt", bufs=2) as kt_pool,
            tc.tile_pool(name="g", bufs=4) as g_pool,
            tc.tile_pool(name="val", bufs=3) as val_pool,
            tc.tile_pool(name="small", bufs=1) as small_pool,
            tc.tile_pool(name="psum", bufs=1, space="PSUM") as psum_pool,
        ):
            # ---------------- constants ----------------
            ident = const_pool.tile([P, P], F32)
            make_identity(nc, ident)
            ones_col = const_pool.tile([P, 1], F32)
            nc.vector.memset(ones_col, 1.0)
            one_sc = const_pool.tile([1, 1], F32)
            nc.vector.memset(one_sc, 1.0)

            # Wk resident in SBUF: [128p, hc, a]  (p is h mod 128)
            wk_sb = const_pool.tile([P, HC, a], mm_dt)
            nc.gpsimd.dma_start(wk_sb, wk_d.rearrange("(hc p) a -> p hc a", p=P))

            # v as [128p, ac]
            v_sb = const_pool.tile([P, AC], mm_dt)
            with nc.allow_non_contiguous_dma(reason="tiny 4KB v load"):
                nc.gpsimd.dma_start(v_sb, v_d.rearrange("(ac p) -> p ac", p=P))

            # queryT: [128p, hc*b] built via PE transpose of [b, 128] blocks
            qraw_sb = const_pool.tile([b_local, h], F32)
            nc.sync.dma_start(qraw_sb, query_d[:, :])
            qt_ps = psum_pool.tile([P, HC * b_local], F32, tag="sc", bufs=2,
                                   name="qt_ps")
            for hc in range(HC):
                nc.tensor.transpose(
                    qt_ps[:, hc * b_local : (hc + 1) * b_local],
                    qraw_sb[:, hc * P : (hc + 1) * P],
                    ident[0:b_local, 0:b_local],
                )
            qt_sb = const_pool.tile([P, HC * b_local], F32)
            nc.vector.tensor_copy(qt_sb, qt_ps)

            # q^T[b, a] = query @ Wq, accumulated over hc.
            q_bt_ps = psum_pool.tile([b_local, a], F32, tag="ctx", bufs=1,
                                     name="q_bt_ps")
            for hc in range(HC):
                wq_sb = wq_pool.tile([P, a], F32)
                nc.sync.dma_start(
                    wq_sb, wq_d.rearrange("(hc p) a -> p hc a", p=P)[:, hc]
                )
                for ha in range(a // 512):
                    nc.tensor.matmul(
                        q_bt_ps[:, ha * 512 : (ha + 1) * 512],
                        lhsT=qt_sb[:, hc * b_local : (hc + 1) * b_local],
                        rhs=wq_sb[:, ha * 512 : (ha + 1) * 512],
                        start=(hc == 0),
                        stop=(hc == HC - 1),
                    )
            q_bt_sb = const_pool.tile([b_local, a], F32)
            nc.scalar.copy(q_bt_sb, q_bt_ps)
            # transpose q^T [b, a] -> q columns [128p(a), ac*b]
            qcol_ps = psum_pool.tile([P, AC * b_local], F32, tag="sc", bufs=2,
                                     name="qcol_ps")
            for ac in range(AC):
                nc.tensor.transpose(
                    qcol_ps[:, ac * b_local : (ac + 1) * b_local],
                    q_bt_sb[:, ac * P : (ac + 1) * P],
                    ident[0:b_local, 0:b_local],
                )
            q_sb = const_pool.tile([P, AC * b_local], F32)
            nc.vector.tensor_copy(q_sb, qcol_ps)

            # ---------------- main loop over local batches ----------------
            for b in range(b_local):
                # p (unnormalized masked softmax numerator) [1, tk],
                # per-chunk entries permuted: idx (c,j) <-> t = t0 + C*j + c
                p_sb = small_pool.tile([1, tk], F32, tag="p")
                masku8_sb = small_pool.tile([1, tk], U8, tag="masku8", bufs=2)
                nc.sync.dma_start(masku8_sb, mask_d[b][None, :])
                # p columns [128t, NJ] for the ctx contraction
                pcol_sb = small_pool.tile([P, NJ], VAL_DT, tag="pcol", bufs=2)
                # ctx accumulator [1, h] lives across the whole batch
                ctx_ps = psum_pool.tile([1, h], F32, tag="ctx", bufs=1)

                def emit_pcol(tci_, val_):
                    t0_ = tci_ * tch
                    pcol_ps = psum_pool.tile([P, C], F32, tag="sc", bufs=2,
                                             name="pcol_ps")
                    for c in range(C):
                        nc.tensor.matmul(
                            pcol_ps[:, c : c + 1],
                            lhsT=p_sb[:, t0_ + c * P : t0_ + (c + 1) * P],
                            rhs=one_sc,
                            start=(c == 0),
                            stop=(c == C - 1),
                        )
                    nc.vector.tensor_copy(
                        pcol_sb[:, tci_ * C : (tci_ + 1) * C], pcol_ps
                    )

                def emit_ctx(tci_, val_):
                    for c in range(C):
                        j = tci_ * C + c
                        for hh in range(h // 512):
                            nc.tensor.matmul(
                                ctx_ps[:, hh * 512 : (hh + 1) * 512],
                                lhsT=pcol_sb[:, j : j + 1],
                                rhs=val_[:, c, hh * 512 : (hh + 1) * 512],
                                start=(j == 0),
                                stop=(j == NJ - 1),
                            )

                prev_chunk = None
                for tci in range(NTCH):
                    t0 = tci * tch
                    # keys chunk [128p, c, h]; DRAM row = t0 + C*p + c so a
                    # partition reads one contiguous C*h*4B run
                    kraw = kraw_pool.tile([P, C, h], F32)
                    nc.sync.dma_start(
                        kraw,
                        keys_d[b, t0 : t0 + tch].rearrange(
                            "(p c) h -> p c h", c=C
                        ),
                    )
                    # values chunk (same permuted layout), streams on SWDGE
                    val = val_pool.tile([P, C, h], VAL_DT)
                    nc.gpsimd.dma_start(
                        val,
                        values_d[b, t0 : t0 + tch].rearrange(
                            "(p c) h -> p c h", c=C
                        ),
                    )
                    # transpose keys to keysT [128p(h), hc, tch]
                    kt = kt_pool.tile([P, HC, tch], mm_dt)
                    for hc in range(HC):
                        tr_ps = psum_pool.tile([P, tch], F32, tag="tr",
                                               bufs=2)
                        for c in range(C):
                            nc.tensor.transpose(
                                tr_ps[:, c * P : (c + 1) * P],
                                kraw[:, c, hc * P : (hc + 1) * P],
                                ident,
                            )
                        nc.vector.tensor_copy(kt[:, hc], tr_ps)

                    # scores^T accumulate [1, tch]; the v^T @ g matmul for
                    # chunk ac is emitted after the main matmuls of ac+1 so
                    # the PE never waits on the ScalarE tanh.
                    sc_ps = psum_pool.tile([1, tch], F32, tag="sc", bufs=2)
                    g_prev = None
                    for ac in range(AC):
                        # flush the previous chunk's pcol / ctx work in the
                        # middle of this chunk's matmul stream, by which time
                        # its exp/mask/copy chain has long finished
                        if ac == 1 and prev_chunk is not None:
                            emit_pcol(*prev_chunk)
                        if ac == 3 and prev_chunk is not None:
                            emit_ctx(*prev_chunk)
                            prev_chunk = None
                        mm_ps = psum_pool.tile([P, tch], F32, tag="mm", bufs=2)
                        for hc in range(HC):
                            nc.tensor.matmul(
                                mm_ps,
                                lhsT=wk_sb[:, hc, ac * P : (ac + 1) * P],
                                rhs=kt[:, hc],
                                start=(hc == 0),
                                stop=(hc == HC - 1),
                            )
                        g = g_pool.tile([P, tch], mm_dt)
                        nc.scalar.activation(
                            g, mm_ps, AF.Tanh,
                            bias=q_sb[:, ac * b_local + b : ac * b_local + b + 1],
                        )
                        if g_prev is not None:
                            nc.tensor.matmul(
                                sc_ps,
                                lhsT=v_sb[:, ac - 1 : ac],
                                rhs=g_prev,
                                start=(ac - 1 == 0),
                                stop=False,
                            )
                        g_prev = g
                    nc.tensor.matmul(
                        sc_ps,
                        lhsT=v_sb[:, AC - 1 : AC],
                        rhs=g_prev,
                        start=False,
                        stop=True,
                    )
                    # p = exp(scores) * mask (mask read un-permutes)
                    nc.scalar.activation(
                        p_sb[:, t0 : t0 + tch], sc_ps, AF.Exp
                    )
                    maskf = small_pool.tile([1, tch], F32, tag="maskf", bufs=2)
                    nc.vector.tensor_copy(
                        maskf.rearrange("o (c j) -> o c j", c=C),
                        masku8_sb[:, t0 : t0 + tch].rearrange(
                            "o (j c) -> o c j", c=C
                        ),
                    )
                    nc.vector.tensor_mul(
                        p_sb[:, t0 : t0 + tch], p_sb[:, t0 : t0 + tch], maskf
                    )

                    prev_chunk = (tci, val)


                if prev_chunk is not None:
                    emit_pcol(*prev_chunk)
                    emit_ctx(*prev_chunk)
                    prev_chunk = None

                # ---- batch epilogue: l, outputs ----
                rowsum = small_pool.tile([P, 1], F32, tag="rowsum", bufs=2)
                nc.vector.tensor_reduce(
                    rowsum, pcol_sb, mybir.AxisListType.X, mybir.AluOpType.add
                )
                l_ps = psum_pool.tile([1, 1], F32, tag="sc", bufs=2)
                nc.tensor.matmul(l_ps, lhsT=rowsum, rhs=ones_col,
                                 start=True, stop=True)
                linv_sb = small_pool.tile([1, 1], F32, tag="linv", bufs=2)
                nc.vector.reciprocal(linv_sb, l_ps)

                # attn output = p * linv (in-place), DVE un-permute, store
                nc.vector.tensor_scalar_mul(p_sb, p_sb, linv_sb)
                attn_sb = small_pool.tile([1, tk], F32, tag="attn", bufs=1)
                nc.vector.tensor_copy(
                    attn_sb.rearrange("o (tc j c) -> o tc j c", c=C, j=P),
                    p_sb.rearrange("o (tc c j) -> o tc j c", c=C, j=P),
                )
                nc.sync.dma_start(attn_d[b][None, :], attn_sb)

                ctx_sb = small_pool.tile([1, h], F32, tag="ctx")
                nc.scalar.activation(ctx_sb, ctx_ps, AF.Copy, scale=linv_sb)
                nc.sync.dma_start(ctx_d[b][None, :], ctx_sb)

    nc.compile()
    return nc


LAST_RESULT = None


def kernel(query, keys, values, mask, Wq, Wk, v, run_kwargs=None):
    global LAST_RESULT
    query = np.asarray(query, dtype=np.float32)
    keys = np.asarray(keys, dtype=np.float32)
    values = np.asarray(values, dtype=np.float32)
    mask_u8 = np.asarray(mask).astype(np.uint8)
    Wq = np.ascontiguousarray(np.asarray(Wq, dtype=np.float32))
    Wk = np.ascontiguousarray(np.asarray(Wk, dtype=np.float32))
    v = np.ascontiguousarray(np.asarray(v, dtype=np.float32))

    nc = build_nc()

    in_maps = []
    for ci in range(N_CORES):
        sl = slice(ci * B_LOCAL, (ci + 1) * B_LOCAL)
        in_maps.append(
            {
                "keys": np.ascontiguousarray(keys[sl]),
                "values": np.ascontiguousarray(values[sl]),
                "query": np.ascontiguousarray(query[sl, 0]),
                "mask": np.ascontiguousarray(mask_u8[sl]),
                "Wq": Wq,
                "Wk": Wk,
                "v": v,
            }
        )

    res = run_bass_kernel_spmd(
        nc, in_maps, core_ids=list(range(N_CORES)), **(run_kwargs or {})
    )
    LAST_RESULT = res
    ctx = np.concatenate([r["ctx"] for r in res.results], axis=0)
    attn = np.concatenate([r["attn"] for r in res.results], axis=0)
    return ctx.reshape(B, TQ, H), attn.reshape(B, TQ, TK)


# revision 30
# speedup vs baseline: 1.3132x; 1.1075x over previous
"""Additive attention (Bahdanau) Trainium2 kernel.

Problem (hardcoded):
    query  [32, 1, 1024] f32
    keys   [32, 4096, 1024] f32
    values [32, 4096, 1024] f32
    mask   [32, 4096] bool
    Wq, Wk [1024, 1024] f32, v [1024] f32

    q = query @ Wq;  k = keys @ Wk
    scores = v . tanh(q + k);  scores = where(mask, scores, -inf)
    attn = softmax(scores);  ctx = attn @ values
    returns (ctx, attn)

Sharding: data-parallel over batch, 4 batches per core on 8 NeuronCores.

Per-core layout ("[a, t]" layout):
  - keys chunks are loaded [128p, c, h] with DRAM row = C*p + c so each
    partition reads one contiguous 16KB run (big DMA descriptors). Within
    a t-chunk the t order is therefore permuted: stored index (c, j)
    corresponds to t = t0 + C*j + c. The permutation is self-consistent
    through scores -> p -> pcol -> ctx (values use the same load layout);
    only the mask read and the attn store un-permute via strided APs.
  - keys tiles [128t, 128h] are PE-transposed to [128h, t] so the H
    contraction sits on partitions.
  - K^T[a, t] = Wk[h,a]^T @ keysT[h,t] accumulated over h-chunks in PSUM.
  - tanh on ScalarE reads PSUM with per-partition bias q[a] (fused add).
  - scores^T[1, t] = v_chunk[128a,1]^T @ g[128a, t] accumulated over a.
  - exp on ScalarE -> p[1, Tk] (no max subtraction needed: |scores|<=32),
    masked by a VectorE multiply.
  - p moved to [128t, Tk/128] via K=1 matmuls; row-reduce + ones-matmul
    give l; ctx = p^T @ values accumulated in PSUM, scaled by 1/l at
    evacuation; attn = p * 1/l on VectorE.
"""

import numpy as np

import concourse.bass as bass
import concourse.tile as tile
from concourse import bacc, mybir
from concourse.bass_utils import run_bass_kernel_spmd
from concourse.masks import make_identity

F32 = mybir.dt.float32
FP8 = mybir.dt.float8e4
F32R = mybir.dt.float32r
BF16 = mybir.dt.bfloat16
U8 = mybir.dt.uint8
AF = mybir.ActivationFunctionType

# Full problem constants
B, TQ, TK, H, A = 32, 1, 4096, 1024, 1024
N_CORES = 8
B_LOCAL = B // N_CORES

# dtype of the keys-projection matmul path:
# "fp8" (e4m3 + DoubleRow) | "bf16" | "f32r" | "f32"
MM_MODE = "fp8"
# dtype of the ctx (attn @ values) matmul path
VAL_DT = F32R

_DT = {"fp8": FP8, "bf16": BF16, "f32r": F32R, "f32": F32}


def build_nc(b_local=B_LOCAL, tk=TK, h=H, a=A, tch=512, mm_mode=None):
    """Build the per-core Bass module (SPMD: same program each core)."""
    mm_dt = _DT[mm_mode or MM_MODE]
    fp8_main = mm_dt == FP8
    g_dt = BF16 if fp8_main else mm_dt
    # Wk is scaled by 16 into e4m3 range; tanh's input scale compensates
    wk_scale = 16.0 if fp8_main else 1.0
    P = 128
    HC = h // P          # h chunks
    AC = a // P          # a chunks
    NTCH = tk // tch     # t chunks
    C = tch // P         # 128-blocks per t chunk
    NJ = tk // P         # t 128-blocks per batch

    nc = bacc.Bacc("TRN2", target_bir_lowering=False, debug=False)

    keys_d = nc.dram_tensor("keys", [b_local, tk, h], F32, kind="ExternalInput")
    values_d = nc.dram_tensor("values", [b_local, tk, h], F32, kind="ExternalInput")
    query_d = nc.dram_tensor("query", [b_local, h], F32, kind="ExternalInput")
    mask_d = nc.dram_tensor("mask", [b_local, tk], U8, kind="ExternalInput")
    wq_d = nc.dram_tensor("Wq", [h, a], F32, kind="ExternalInput")
    wk_d = nc.dram_tensor("Wk", [h, a], F32, kind="ExternalInput")
    v_d = nc.dram_tensor("v", [a], F32, kind="ExternalInput")

    ctx_d = nc.dram_tensor("ctx", [b_local, h], F32, kind="ExternalOutput")
    attn_d = nc.dram_tensor("attn", [b_local, tk], F32, kind="ExternalOutput")

    with tile.TileContext(nc) as tc:
        with (
            tc.tile_pool(name="const", bufs=1) as const_pool,
            tc.tile_pool(name="wq", bufs=2) as wq_pool,
            tc.tile_pool(name="kraw", bufs=3) as kraw_pool,
            tc.tile_pool(name="kt", bufs=2) as kt_pool,
            tc.tile_pool(name="g", bufs=4) as g_pool,
            tc.tile_pool(name="val", bufs=3) as val_pool,
            tc.tile_pool(name="small", bufs=1) as small_pool,
            tc.tile_pool(name="psum", bufs=1, space="PSUM") as psum_pool,
        ):
            # ---------------- constants ----------------
            ident = const_pool.tile([P, P], F32)
            make_identity(nc, ident)
            ones_col = const_pool.tile([P, 1], F32)
            nc.vector.memset(ones_col, 1.0)
            one_sc = const_pool.tile([1, 1], F32)
            nc.vector.memset(one_sc, 1.0)

            # Wk resident in SBUF: [128p, hc, a]  (p is h mod 128)
            wk_sb = const_pool.tile([P, HC, a], mm_dt)
            if fp8_main:
                for hc in range(HC):
                    wk_stage = wq_pool.tile([P, a], F32, name="wk_stage")
                    nc.sync.dma_start(
                        wk_stage, wk_d.rearrange("(hc p) a -> p hc a", p=P)[:, hc]
                    )
                    nc.scalar.activation(
                        wk_sb[:, hc], wk_stage, AF.Copy, scale=wk_scale
                    )
            else:
                nc.gpsimd.dma_start(
                    wk_sb, wk_d.rearrange("(hc p) a -> p hc a", p=P)
                )

            # v as [128p, ac]
            v_sb = const_pool.tile([P, AC], g_dt)
            with nc.allow_non_contiguous_dma(reason="tiny 4KB v load"):
                nc.gpsimd.dma_start(v_sb, v_d.rearrange("(ac p) -> p ac", p=P))

            # queryT: [128p, hc*b] built via PE transpose of [b, 128] blocks
            qraw_sb = const_pool.tile([b_local, h], F32)
            nc.sync.dma_start(qraw_sb, query_d[:, :])
            qt_ps = psum_pool.tile([P, HC * b_local], F32, tag="sc", bufs=2,
                                   name="qt_ps")
            for hc in range(HC):
                nc.tensor.transpose(
                    qt_ps[:, hc * b_local : (hc + 1) * b_local],
                    qraw_sb[:, hc * P : (hc + 1) * P],
                    ident[0:b_local, 0:b_local],
                )
            qt_sb = const_pool.tile([P, HC * b_local], F32)
            nc.vector.tensor_copy(qt_sb, qt_ps)

            # q^T[b, a] = query @ Wq, accumulated over hc.
            q_bt_ps = psum_pool.tile([b_local, a], F32, tag="ctx", bufs=1,
                                     name="q_bt_ps")
            for hc in range(HC):
                wq_sb = wq_pool.tile([P, a], F32)
                nc.sync.dma_start(
                    wq_sb, wq_d.rearrange("(hc p) a -> p hc a", p=P)[:, hc]
                )
                for ha in range(a // 512):
                    nc.tensor.matmul(
                        q_bt_ps[:, ha * 512 : (ha + 1) * 512],
                        lhsT=qt_sb[:, hc * b_local : (hc + 1) * b_local],
                        rhs=wq_sb[:, ha * 512 : (ha + 1) * 512],
                        start=(hc == 0),
                        stop=(hc == HC - 1),
                    )
            q_bt_sb = const_pool.tile([b_local, a], F32)
            nc.scalar.copy(q_bt_sb, q_bt_ps)
            # transpose q^T [b, a] -> q columns [128p(a), ac*b]
            qcol_ps = psum_pool.tile([P, AC * b_local], F32, tag="sc", bufs=2,
                                     name="qcol_ps")
            for ac in range(AC):
                nc.tensor.transpose(
                    qcol_ps[:, ac * b_local : (ac + 1) * b_local],
                    q_bt_sb[:, ac * P : (ac + 1) * P],
                    ident[0:b_local, 0:b_local],
                )
            q_sb = const_pool.tile([P, AC * b_local], F32)
            nc.vector.tensor_copy(q_sb, qcol_ps)

            # ---------------- main loop over local batches ----------------
            for b in range(b_local):
                # p (unnormalized masked softmax numerator) [1, tk],
                # per-chunk entries permuted: idx (c,j) <-> t = t0 + C*j + c
                p_sb = small_pool.tile([1, tk], F32, tag="p")
                masku8_sb = small_pool.tile([1, tk], U8, tag="masku8", bufs=2)
                nc.sync.dma_start(masku8_sb, mask_d[b][None, :])
                # p columns [128t, NJ] for the ctx contraction
                pcol_sb = small_pool.tile([P, NJ], VAL_DT, tag="pcol", bufs=2)
                # ctx accumulator [1, h] lives across the whole batch
                ctx_ps = psum_pool.tile([1, h], F32, tag="ctx", bufs=1)

                def emit_pcol(tci_, val_):
                    t0_ = tci_ * tch
                    pcol_ps = psum_pool.tile([P, C], F32, tag="sc", bufs=2,
                                             name="pcol_ps")
                    for c in range(C):
                        nc.tensor.matmul(
                            pcol_ps[:, c : c + 1],
                            lhsT=p_sb[:, t0_ + c * P : t0_ + (c + 1) * P],
                            rhs=one_sc,
                            start=(c == 0),
                            stop=(c == C - 1),
                        )
                    nc.vector.tensor_copy(
                        pcol_sb[:, tci_ * C : (tci_ + 1) * C], pcol_ps
                    )

                def emit_ctx(tci_, val_):
                    for c in range(C):
                        j = tci_ * C + c
                        for hh in range(h // 512):
                            nc.tensor.matmul(
                                ctx_ps[:, hh * 512 : (hh + 1) * 512],
                                lhsT=pcol_sb[:, j : j + 1],
                                rhs=val_[:, c, hh * 512 : (hh + 1) * 512],
                                start=(j == 0),
                                stop=(j == NJ - 1),
                            )

                prev_chunk = None
                for tci in range(NTCH):
                    t0 = tci * tch
                    # keys chunk [128p, c, h]; DRAM row = t0 + C*p + c so a
                    # partition reads one contiguous C*h*4B run
                    kraw = kraw_pool.tile([P, C, h], F32)
                    nc.sync.dma_start(
                        kraw,
                        keys_d[b, t0 : t0 + tch].rearrange(
                            "(p c) h -> p c h", c=C
                        ),
                    )
                    # values chunk (same permuted layout), streams on SWDGE
                    val = val_pool.tile([P, C, h], VAL_DT)
                    nc.gpsimd.dma_start(
                        val,
                        values_d[b, t0 : t0 + tch].rearrange(
                            "(p c) h -> p c h", c=C
                        ),
                    )
                    # transpose keys to keysT [128p(h), hc, tch]
                    kt = kt_pool.tile([P, HC, tch], mm_dt)
                    for hc in range(HC):
                        tr_ps = psum_pool.tile([P, tch], F32, tag="tr",
                                               bufs=2)
                        for c in range(C):
                            nc.tensor.transpose(
                                tr_ps[:, c * P : (c + 1) * P],
                                kraw[:, c, hc * P : (hc + 1) * P],
                                ident,
                            )
                        nc.vector.tensor_copy(kt[:, hc], tr_ps)

                    # scores^T accumulate [1, tch]; the v^T @ g matmul for
                    # chunk ac is emitted after the main matmuls of ac+1 so
                    # the PE never waits on the ScalarE tanh.
                    sc_ps = psum_pool.tile([1, tch], F32, tag="sc", bufs=2)
                    g_prev = None
                    for ac in range(AC):
                        # flush the previous chunk's pcol / ctx work in the
                        # middle of this chunk's matmul stream, by which time
                        # its exp/mask/copy chain has long finished
                        if ac == 1 and prev_chunk is not None:
                            emit_pcol(*prev_chunk)
                        if ac == 3 and prev_chunk is not None:
                            emit_ctx(*prev_chunk)
                            prev_chunk = None
                        mm_ps = psum_pool.tile([P, tch], F32, tag="mm", bufs=2)
                        if fp8_main:
                            for hc in range(0, HC, 2):
                                nc.tensor.matmul(
                                    mm_ps,
                                    lhsT=wk_sb[:, hc : hc + 2,
                                               ac * P : (ac + 1) * P],
                                    rhs=kt[:, hc : hc + 2],
                                    start=(hc == 0),
                                    stop=(hc == HC - 2),
                                    perf_mode=mybir.MatmulPerfMode.DoubleRow,
                                )
                        else:
                            for hc in range(HC):
                                nc.tensor.matmul(
                                    mm_ps,
                                    lhsT=wk_sb[:, hc, ac * P : (ac + 1) * P],
                                    rhs=kt[:, hc],
                                    start=(hc == 0),
                                    stop=(hc == HC - 1),
                                )
                        g = g_pool.tile([P, tch], g_dt)
                        nc.scalar.activation(
                            g, mm_ps, AF.Tanh,
                            bias=q_sb[:, ac * b_local + b : ac * b_local + b + 1],
                            scale=1.0 / wk_scale,
                        )
                        if g_prev is not None:
                            nc.tensor.matmul(
                                sc_ps,
                                lhsT=v_sb[:, ac - 1 : ac],
                                rhs=g_prev,
                                start=(ac - 1 == 0),
                                stop=False,
                            )
                        g_prev = g
                    nc.tensor.matmul(
                        sc_ps,
                        lhsT=v_sb[:, AC - 1 : AC],
                        rhs=g_prev,
                        start=False,
                        stop=True,
                    )
                    # p = exp(scores) * mask (mask read un-permutes)
                    nc.scalar.activation(
                        p_sb[:, t0 : t0 + tch], sc_ps, AF.Exp
                    )
                    maskf = small_pool.tile([1, tch], F32, tag="maskf", bufs=2)
                    nc.vector.tensor_copy(
                        maskf.rearrange("o (c j) -> o c j", c=C),
                        masku8_sb[:, t0 : t0 + tch].rearrange(
                            "o (j c) -> o c j", c=C
                        ),
                    )
                    nc.vector.tensor_mul(
                        p_sb[:, t0 : t0 + tch], p_sb[:, t0 : t0 + tch], maskf
                    )

                    prev_chunk = (tci, val)


                if prev_chunk is not None:
                    emit_pcol(*prev_chunk)
                    emit_ctx(*prev_chunk)
                    prev_chunk = None

                # ---- batch epilogue: l, outputs ----
                rowsum = small_pool.tile([P, 1], F32, tag="rowsum", bufs=2)
                nc.vector.tensor_reduce(
                    rowsum, pcol_sb, mybir.AxisListType.X, mybir.AluOpType.add
                )
                l_ps = psum_pool.tile([1, 1], F32, tag="sc", bufs=2)
                nc.tensor.matmul(l_ps, lhsT=rowsum, rhs=ones_col,
                                 start=True, stop=True)
                linv_sb = small_pool.tile([1, 1], F32, tag="linv", bufs=2)
                nc.vector.reciprocal(linv_sb, l_ps)

                # attn output = p * linv (in-place), DVE un-permute, store
                nc.vector.tensor_scalar_mul(p_sb, p_sb, linv_sb)
                attn_sb = small_pool.tile([1, tk], F32, tag="attn", bufs=1)
                nc.vector.tensor_copy(
                    attn_sb.rearrange("o (tc j c) -> o tc j c", c=C, j=P),
                    p_sb.rearrange("o (tc c j) -> o tc j c", c=C, j=P),
                )
                nc.sync.dma_start(attn_d[b][None, :], attn_sb)

                ctx_sb = small_pool.tile([1, h], F32, tag="ctx")
                nc.scalar.activation(ctx_sb, ctx_ps, AF.Copy, scale=linv_sb)
                nc.sync.dma_start(ctx_d[b][None, :], ctx_sb)

    nc.compile()
    return nc


LAST_RESULT = None


def kernel(query, keys, values, mask, Wq, Wk, v, run_kwargs=None):
    global LAST_RESULT
    query = np.asarray(query, dtype=np.float32)
    keys = np.asarray(keys, dtype=np.float32)
    values = np.asarray(values, dtype=np.float32)
    mask_u8 = np.asarray(mask).astype(np.uint8)
    Wq = np.ascontiguousarray(np.asarray(Wq, dtype=np.float32))
    Wk = np.ascontiguousarray(np.asarray(Wk, dtype=np.float32))
    v = np.ascontiguousarray(np.asarray(v, dtype=np.float32))

    nc = build_nc()

    in_maps = []
    for ci in range(N_CORES):
        sl = slice(ci * B_LOCAL, (ci + 1) * B_LOCAL)
        in_maps.append(
            {
                "keys": np.ascontiguousarray(keys[sl]),
                "values": np.ascontiguousarray(values[sl]),
                "query": np.ascontiguousarray(query[sl, 0]),
                "mask": np.ascontiguousarray(mask_u8[sl]),
                "Wq": Wq,
                "Wk": Wk,
                "v": v,
            }
        )

    res = run_bass_kernel_spmd(
        nc, in_maps, core_ids=list(range(N_CORES)), **(run_kwargs or {})
    )
    LAST_RESULT = res
    ctx = np.concatenate([r["ctx"] for r in res.results], axis=0)
    attn = np.concatenate([r["attn"] for r in res.results], axis=0)
    return ctx.reshape(B, TQ, H), attn.reshape(B, TQ, TK)
